# revision 29
# baseline (speedup 1.0000x reference)
"""Trainium2 Bass kernel for nn_BoxRepelLoss (rotated-box repel/IoU loss).

Math: replaces the reference's convex-hull-by-argsort intersection area with
an equivalent sort-free Green's-theorem form. For convex CCW polygons P, Q:

    2*Area(P inter Q) = sum over the 8 edges (4 of P Liang-Barsky-clipped
    against Q's slab half-planes, 4 of Q against P's) of
    (t_hi - t_lo) * cross(a, b - a),  t clamped to [0, 1]

since each clipped segment's line-integral contribution collapses to
dt * cross(a, e). All per-pair work is elementwise -> Vector engine.

Pair enumeration (halves work vs the full [m, m] grid): unordered pairs
(i, (i+k) mod m), k = 1..m/2; the k = m/2 row appears on two cores and is
weighted 0.5 on both (bitwise-identical values, so the sum stays exact).
Grid layout per core: partition p x free (kt, c), with k = kt*128 + p + 1
(kt = 0..2) and i = 96*d + c (c = 0..95) -- core d owns a 96-column i-slab.
Features reach each core as:
  - "peri" [NR, 288]      per-i rows (pre-replicated x3), partition-broadcast
  - "hank" [NR*3, 224]    sliding windows of the wrap-extended feature arrays;
                          partner j = i + k features materialize via Hankel
                          access patterns f[p + 1 + kt*128 + c]
Both directions' edge quantities live in one 8-slot [128, 8*288] layout
(slots = 4 edges x 2 directions) so the Liang-Barsky interval math runs as
~2300-wide DVE ops. Edge projections come from corner-projection differences
(r[e] = dca[(e+1)%4] - dca[e]); interval endpoints use
lo = -w2*|1/r| - dca/r, hi = +w2*|1/r| - dca/r (no root sort needed).

Each core emits partial sums (S_iou, S_rep, S_size); the host combines:
  total = 2*S_rep/(m(m-1)) + S_size/m + 2*S_iou/m^2
"""

import numpy as np

M = 768
NDEV = 8
CPD = M // NDEV          # 96 i-columns per core
NKT = 3                  # k-tiles: k = kt*128 + p + 1 in [1, 384]
W288 = NKT * CPD         # 288 pair-columns per partition
W1152 = 4 * W288         # one direction, 4 edge slots
W2304 = 2 * W1152        # both directions
HROW = 224               # hankel window row length (per (kt,r) row)

# feature-row indices (same semantics in peri and hank)
R_XA, R_YA, R_K = 0, 4, 8
R_COS, R_SIN, R_UC, R_US, R_W2, R_H2 = 12, 13, 14, 15, 16, 17
R_CX, R_CY, R_A2, R_WCOL = 18, 19, 20, 21
NR = 22

REPEL_MARGIN = 0.08
MIN_SIZE = 0.02
IOU_MARGIN = 0.1

_PROGRAM_CACHE = {}


def _features(pred):
    """Per-box feature table F [NR-1, M] (fp32, matching reference math)."""
    p = np.asarray(pred, np.float32)[:-1]
    cx, cy, w, h = p[:, 0], p[:, 1], p[:, 2], p[:, 3]
    th = np.arctan2(p[:, 5], p[:, 4]).astype(np.float32)
    c = np.cos(th).astype(np.float32)
    s = np.sin(th).astype(np.float32)
    dx = np.stack([-w, w, w, -w], 0) * np.float32(0.5)   # [4, M]
    dy = np.stack([-h, -h, h, h], 0) * np.float32(0.5)
    xa = cx[None] + c[None] * dx - s[None] * dy           # [4, M]
    ya = cy[None] + s[None] * dx + c[None] * dy
    ex = np.roll(xa, -1, 0) - xa
    ey = np.roll(ya, -1, 0) - ya
    K = xa * ey - ya * ex
    F = np.empty((NR - 1, M), np.float32)
    F[R_XA:R_XA + 4] = xa
    F[R_YA:R_YA + 4] = ya
    F[R_K:R_K + 4] = K
    F[R_COS], F[R_SIN] = c, s
    F[R_UC] = c * cx + s * cy
    F[R_US] = -s * cx + c * cy
    F[R_W2], F[R_H2] = w * 0.5, h * 0.5
    F[R_CX], F[R_CY] = cx, cy
    F[R_A2] = 2.0 * w * h
    return F


# DMA row groups in consumption order: the first A-phase ops need only
# cos/sin/uc/us (clip) + xa/ya (subject); w2..wcol feed B and the epilogue;
# K rows are only needed by the C phase.
_GROUPS = [(R_COS, R_W2), (R_XA, R_K), (R_W2, NR), (R_K, R_COS)]


def _build_program():
    import concourse.bass as bass
    import concourse.mybir as mybir
    from concourse import bacc
    from concourse.tile import TileContext

    fp32 = mybir.dt.float32
    Alu = mybir.AluOpType
    Act = mybir.ActivationFunctionType

    nc = bacc.Bacc('TRN2', target_bir_lowering=False, debug=False)
    for v in (REPEL_MARGIN, MIN_SIZE):
        t = nc.alloc_sbuf_tensor(f'const-f32-{v}', [128, 1], fp32)
        nc.gpsimd.memset(t.ap(), v)
        nc.const_aps.aps[(fp32, v)] = t.ap()
    nc.all_engine_barrier()

    hank = nc.dram_tensor('hank', [NR * NKT, HROW], fp32, kind='ExternalInput')
    peri = nc.dram_tensor('peri', [NR, W288], fp32, kind='ExternalInput')
    out = nc.dram_tensor('out', [4, 1], fp32, kind='ExternalOutput')

    def sub(t, off, free_dims):
        base = t[:]
        return bass.AP(base.tensor, base.offset + off, [list(base.ap[0])] + free_dims)

    with TileContext(nc) as tc:
        with tc.tile_pool(name='p', bufs=1) as pool, \
             tc.tile_pool(name='ps', bufs=1, space='PSUM') as ppool:
            psum4 = ppool.tile([4, 1], fp32, tag='psum4')
            hank_sb = pool.tile([128, NR * W288], fp32, tag='hank')
            peri_sb = pool.tile([128, NR * W288], fp32, tag='peri')

            hout, pout = hank_sb[:], peri_sb[:]
            for (a, b) in _GROUPS:
                n = b - a
                nc.sync.dma_start(
                    out=bass.AP(hout.tensor, hout.offset + a * W288,
                                [list(hout.ap[0]), [CPD, n * NKT], [1, CPD]]),
                    in_=bass.AP(hank[:].tensor, a * NKT * HROW + 1,
                                [[1, 128], [HROW, n * NKT], [1, CPD]]))
                nc.sync.dma_start(
                    out=bass.AP(pout.tensor, pout.offset + a * W288,
                                [list(pout.ap[0]), [1, n * W288]]),
                    in_=bass.AP(peri[:].tensor, a * W288,
                                [[0, 128], [1, n * W288]]))

            def crow(bank, r):   # clip row, e-broadcast [128, 4, 288]
                return sub(bank, r * W288, [[0, 4], [1, W288]])

            def v4(bank, r0):    # 4-row block as [128, 4, 288]
                return sub(bank, r0 * W288, [[W288, 4], [1, W288]])

            def flat4(bank, r0):  # 4-row block as [128, 1152]
                return sub(bank, r0 * W288, [[1, W1152]])

            def frow(bank, r):   # single row [128, 288]
                return sub(bank, r * W288, [[1, W288]])

            wcol = sub(hank_sb, R_WCOL * W288, [[1, 1]])

            def wt(tag):
                return pool.tile([128, W2304], fp32, tag=tag, name=tag)

            dca_c, dca_s = wt('dca_c'), wt('dca_s')
            r_c, r_s = wt('r_c'), wt('r_s')
            scr, t1, t2 = wt('scr'), wt('t1'), wt('t2')
            S = pool.tile([128, W288], fp32, tag='S')
            U = pool.tile([128, W288], fp32, tag='U')
            R = pool.tile([128, W288], fp32, tag='R')
            X1 = pool.tile([128, W288], fp32, tag='X1')
            X2 = pool.tile([128, W288], fp32, tag='X2')
            z96a = pool.tile([1, CPD], fp32, tag='z96a')
            z96b = pool.tile([1, CPD], fp32, tag='z96b')
            acc4 = pool.tile([128, 4], fp32, tag='acc4')
            red4 = pool.tile([128, 4], fp32, tag='red4')
            ones = pool.tile([128, 1], fp32, tag='ones')

            tt = nc.vector.tensor_tensor
            ts = nc.vector.tensor_scalar
            stt = nc.vector.scalar_tensor_tensor

            def half4(t, ho):    # one direction half viewed [128, 4, 288]
                return sub(t, ho, [[W288, 4], [1, W288]])

            def seg(t, lo, hi):  # flat column range
                return t[:, lo:hi]

            # ---- A phase: corner projections + edge projections ----
            # (measured: GpSimd TT is ~3x slower than DVE here, so offloading
            # one direction to it lengthens the critical path -- keep all DVE)
            for ho, subj, clip in ((0, peri_sb, hank_sb), (W1152, hank_sb, peri_sb)):
                Cc, Cs = crow(clip, R_COS), crow(clip, R_SIN)
                tt(out=half4(scr, ho), in0=Cc, in1=v4(subj, R_XA), op=Alu.mult)
                tt(out=half4(t1, ho), in0=Cs, in1=v4(subj, R_YA), op=Alu.mult)
                tt(out=seg(scr, ho, ho + W1152), in0=seg(scr, ho, ho + W1152),
                   in1=seg(t1, ho, ho + W1152), op=Alu.add)
                tt(out=half4(dca_c, ho), in0=half4(scr, ho),
                   in1=crow(clip, R_UC), op=Alu.subtract)
                tt(out=half4(scr, ho), in0=Cc, in1=v4(subj, R_YA), op=Alu.mult)
                tt(out=half4(t1, ho), in0=Cs, in1=v4(subj, R_XA), op=Alu.mult)
                tt(out=seg(scr, ho, ho + W1152), in0=seg(scr, ho, ho + W1152),
                   in1=seg(t1, ho, ho + W1152), op=Alu.subtract)
                tt(out=half4(dca_s, ho), in0=half4(scr, ho),
                   in1=crow(clip, R_US), op=Alu.subtract)
                # edge projections r[e] = dca[(e+1)%4] - dca[e]
                for dca, rr in ((dca_c, r_c), (dca_s, r_s)):
                    tt(out=seg(rr, ho, ho + 3 * W288),
                       in0=seg(dca, ho + W288, ho + W1152),
                       in1=seg(dca, ho, ho + 3 * W288), op=Alu.subtract)
                    tt(out=seg(rr, ho + 3 * W288, ho + W1152),
                       in0=seg(dca, ho, ho + W288),
                       in1=seg(dca, ho + 3 * W288, ho + W1152), op=Alu.subtract)

            # ---- B phase (both directions fused, 2304-wide) ----
            # h = w2 * rinv; habs = max(h, -h); hi = habs - g; lo = -habs - g
            for dca, rr, w2r, habs, lo_dst in (
                    (dca_c, r_c, R_W2, t2, t2),
                    (dca_s, r_s, R_H2, dca_c, t1)):
                nc.vector.reciprocal_approx_fast(out=t1[:], in_=rr[:])
                tt(out=scr[:], in0=dca[:], in1=t1[:], op=Alu.mult)
                tt(out=half4(rr, 0), in0=crow(hank_sb, w2r),
                   in1=half4(t1, 0), op=Alu.mult)
                tt(out=half4(rr, W1152), in0=crow(peri_sb, w2r),
                   in1=half4(t1, W1152), op=Alu.mult)
                stt(out=habs[:], in0=rr[:], scalar=-1.0, in1=rr[:],
                    op0=Alu.mult, op1=Alu.max)
                tt(out=rr[:], in0=habs[:], in1=scr[:], op=Alu.subtract)
                stt(out=lo_dst[:], in0=habs[:], scalar=-1.0, in1=scr[:],
                    op0=Alu.mult, op1=Alu.subtract)

            # ---- C phase: clamp, dt, weight by cross const, reduce ----
            tt(out=t1[:], in0=t2[:], in1=t1[:], op=Alu.max)        # LO
            ts(out=t1[:], in0=t1[:], scalar1=0.0, scalar2=1.0,
               op0=Alu.max, op1=Alu.min)
            tt(out=r_c[:], in0=r_c[:], in1=r_s[:], op=Alu.min)     # HI
            ts(out=r_c[:], in0=r_c[:], scalar1=0.0, scalar2=1.0,
               op0=Alu.max, op1=Alu.min)
            tt(out=t1[:], in0=r_c[:], in1=t1[:], op=Alu.subtract)  # dt
            ts(out=t1[:], in0=t1[:], scalar1=0.0, scalar2=None, op0=Alu.max)
            tt(out=seg(t1, 0, W1152), in0=seg(t1, 0, W1152),
               in1=flat4(peri_sb, R_K), op=Alu.mult)
            tt(out=seg(t1, W1152, W2304), in0=seg(t1, W1152, W2304),
               in1=flat4(hank_sb, R_K), op=Alu.mult)
            tt(out=seg(t1, 0, W1152), in0=seg(t1, 0, W1152),
               in1=seg(t1, W1152, W2304), op=Alu.add)
            tt(out=seg(t1, 0, 2 * W288), in0=seg(t1, 0, 2 * W288),
               in1=seg(t1, 2 * W288, W1152), op=Alu.add)
            tt(out=S[:], in0=seg(t1, 0, W288), in1=seg(t1, W288, 2 * W288),
               op=Alu.add)

            # ---- IoU epilogue ----
            tt(out=U[:], in0=frow(peri_sb, R_A2), in1=frow(hank_sb, R_A2),
               op=Alu.add)
            tt(out=U[:], in0=U[:], in1=S[:], op=Alu.subtract)      # union2
            nc.vector.reciprocal_approx_fast(out=R[:], in_=U[:])
            tt(out=R[:], in0=S[:], in1=R[:], op=Alu.mult)          # iou
            ts(out=R[:], in0=R[:], scalar1=IOU_MARGIN, scalar2=0.0,
               op0=Alu.subtract, op1=Alu.max)
            nc.vector.memset(acc4[:], 0.0)
            ts(out=R[:, 2 * CPD:W288], in0=R[:, 2 * CPD:W288],
               scalar1=wcol, scalar2=None, op0=Alu.mult)
            nc.vector.tensor_reduce(out=acc4[:, 0:1], in_=R[:],
                                    axis=mybir.AxisListType.X, op=Alu.add)

            # ---- repel ----
            tt(out=X1[:], in0=frow(hank_sb, R_CX), in1=frow(peri_sb, R_CX),
               op=Alu.subtract)
            tt(out=X2[:], in0=frow(hank_sb, R_CY), in1=frow(peri_sb, R_CY),
               op=Alu.subtract)
            tt(out=X1[:], in0=X1[:], in1=X1[:], op=Alu.mult)
            tt(out=X2[:], in0=X2[:], in1=X2[:], op=Alu.mult)
            tt(out=X1[:], in0=X1[:], in1=X2[:], op=Alu.add)
            nc.scalar.activation(out=X1[:], in_=X1[:], func=Act.Sqrt)
            nc.scalar.activation(out=X1[:], in_=X1[:], func=Act.Relu,
                                 bias=REPEL_MARGIN, scale=-1.0)
            ts(out=X1[:, 2 * CPD:W288], in0=X1[:, 2 * CPD:W288],
               scalar1=wcol, scalar2=None, op0=Alu.mult)
            nc.vector.tensor_reduce(out=acc4[:, 1:2], in_=X1[:],
                                    axis=mybir.AxisListType.X, op=Alu.add)

            # ---- size penalty (this core's 96 boxes) ----
            nc.scalar.activation(out=z96a[:],
                                 in_=peri_sb[0:1, R_W2 * W288:R_W2 * W288 + CPD],
                                 func=Act.Relu, bias=MIN_SIZE, scale=-2.0)
            nc.scalar.activation(out=z96b[:],
                                 in_=peri_sb[0:1, R_H2 * W288:R_H2 * W288 + CPD],
                                 func=Act.Relu, bias=MIN_SIZE, scale=-2.0)
            tt(out=z96a[:], in0=z96a[:], in1=z96b[:], op=Alu.add)
            nc.vector.tensor_reduce(out=acc4[0:1, 2:3], in_=z96a[:],
                                    axis=mybir.AxisListType.X, op=Alu.add)

            # ---- partition reduction via PE, then DMA out ----
            nc.vector.memset(ones[:], 1.0)
            nc.tensor.matmul(out=psum4[:], lhsT=acc4[:], rhs=ones[:],
                             start=True, stop=True)
            nc.scalar.activation(out=red4[0:4, 0:1], in_=psum4[:], func=Act.Copy)
            nc.sync.dma_start(out=out[:], in_=red4[0:4, 0:1])
    nc.compile()
    return nc


def _prep_inputs(pred):
    F = _features(pred)                           # [NR-1, M]
    Fe = np.concatenate([F, F[:, :M // 2]], 1)    # wrap-extended
    in_maps = []
    for d in range(NDEV):
        hank2 = np.empty((NR * NKT, HROW), np.float32)
        for r in range(NR - 1):
            for kt in range(NKT):
                base = d * CPD + 128 * kt
                hank2[r * NKT + kt] = Fe[r, base:base + HROW]
        wrow = np.ones(HROW, np.float32)
        wrow[128] = 0.5          # partition 127 reads Row[1+127]: k=384 dup
        for kt in range(NKT):
            hank2[R_WCOL * NKT + kt] = wrow
        peri2 = np.tile(
            np.vstack([F, np.zeros((1, M), np.float32)])[:, d * CPD:(d + 1) * CPD],
            (1, NKT))
        in_maps.append({'peri': np.ascontiguousarray(peri2), 'hank': hank2})
    return in_maps


def _combine(partials):
    m = float(M)
    S_iou = sum(float(p[0, 0]) for p in partials)
    S_rep = sum(float(p[1, 0]) for p in partials)
    S_size = sum(float(p[2, 0]) for p in partials)
    return np.array((2.0 * S_rep) / (m * (m - 1.0)) + S_size / m
                    + (2.0 * S_iou) / (m * m), dtype=np.float32)


def kernel(pred):
    from concourse import bass_utils
    if 'nc' not in _PROGRAM_CACHE:
        _PROGRAM_CACHE['nc'] = _build_program()
    nc = _PROGRAM_CACHE['nc']
    in_maps = _prep_inputs(pred)
    res = bass_utils.run_bass_kernel_spmd(nc, in_maps, core_ids=list(range(NDEV)))
    return _combine([r['out'] for r in res.results])


if __name__ == '__main__':
    pred = np.load('/root/problem/pred.npy')
    print('kernel total:', kernel(pred))
